# revision 4
# baseline (speedup 1.0000x reference)
"""Viterbi CRF decode on 8 Trainium2 NeuronCores.

Strategy: data-parallel over batch (32 sequences/core). The device kernel runs
the forward max-plus DP (alpha recurrence, the dominant compute) and streams the
full alpha history back to HBM. The host then does the O(L*B*T) backtrack over
that history (0.03% of the FLOPs) plus the sequence-length freeze handling.

Exactness: the device computes alpha_t[j] = max_i(fp32(alpha_{t-1}[i] +
trans[i,j])) + pot_t[j] with the same fp32 rounding as the jax reference, so the
backtrack argmax decisions (first-index tie-break) match bitwise.

Device layout per step (128 partitions = 4 j-quadrants x 32 sequences):
  vt[(q,b), (jb,i)] = alpha[b,i] + trans[i, 16q+jb]   (broadcast add, 1024/partition)
  m4[(q,b), jb]     = max_i vt                        (free-dim reduce)
  alpha'[b, 16q+jb] = m4[(q,b), jb] + pot             (4 collapse copies + add)
"""

import numpy as np

B, L, T = 256, 1024, 64
NCORES = 8
BC = B // NCORES  # 32 sequences per core
CH = 128          # potentials chunk (steps per DMA)

_cache = {}


def _build_program():
    if "nc" in _cache:
        return _cache["nc"]
    import concourse.bacc as bacc
    import concourse.mybir as mybir
    from concourse.tile import TileContext

    f32 = mybir.dt.float32
    AX = mybir.AxisListType
    OP = mybir.AluOpType

    nc = bacc.Bacc("TRN2", target_bir_lowering=False, debug=False)
    pots_in = nc.dram_tensor("pots", [BC, L, T], f32, kind="ExternalInput").ap()
    tsp_in = nc.dram_tensor("tspread", [4, 16, T], f32, kind="ExternalInput").ap()
    sq_in = nc.dram_tensor("selq", [4, 128], f32, kind="ExternalInput").ap()
    sb_in = nc.dram_tensor("selb", [BC, 128], f32, kind="ExternalInput").ap()
    hist_out = nc.dram_tensor("ahist", [BC, L, T], f32, kind="ExternalOutput").ap()

    with TileContext(nc) as tc:
        with tc.tile_pool(name="const", bufs=1) as cpool, \
             tc.tile_pool(name="pstream", bufs=2) as ppool, \
             tc.tile_pool(name="work", bufs=3) as wpool, \
             tc.tile_pool(name="psum", bufs=2, space="PSUM") as psp, \
             tc.tile_pool(name="big", bufs=1) as bpool:
            t4 = cpool.tile([4, 16, T], f32)
            nc.gpsimd.dma_start(out=t4[:], in_=tsp_in[:])
            sq = cpool.tile([4, 128], f32)
            nc.gpsimd.dma_start(out=sq[:], in_=sq_in[:])
            sb = cpool.tile([BC, 128], f32)
            nc.gpsimd.dma_start(out=sb[:], in_=sb_in[:])
            hist = bpool.tile([128, 256, T], f32)   # alpha history, 64KB/partition

            alpha_tiles = {}

            nchunks = L // CH
            for c in range(nchunks):
                pc = ppool.tile([BC, CH, T], f32, tag="pots")
                nc.gpsimd.dma_start(out=pc[:], in_=pots_in[:, c * CH:(c + 1) * CH, :])

                if c == 0:
                    a0 = wpool.tile([BC, T], f32, tag="alpha")
                    nc.vector.tensor_copy(a0[:], pc[:, 0, :])
                    nc.gpsimd.tensor_copy(hist[0:BC, 0, :], a0[:])
                    alpha_tiles[0] = a0

                t0 = max(c * CH, 1)
                for t in range(t0, (c + 1) * CH):
                    tg, tl = t >> 8, t & 255
                    s = t - c * CH
                    alpha = alpha_tiles.pop(t - 1)
                    # vt[p, jb, i] = trans[i, 16*(p//32)+jb] + alpha[p%32, i]  (PE, exact)
                    psum = psp.tile([128, 16, T], f32, tag="vt")
                    for h in range(2):
                        sl = slice(8 * h, 8 * (h + 1))
                        nc.tensor.matmul(psum[:, sl, :], sq[:], t4[:, sl, :],
                                         start=True, stop=False)
                        nc.tensor.matmul(psum[:, sl, :], sb[:],
                                         alpha[:].unsqueeze(1).broadcast_to([BC, 8, T]),
                                         start=False, stop=True)
                    m4 = wpool.tile([128, 16], f32, tag="m4")
                    nc.vector.tensor_reduce(m4[:], psum[:], axis=AX.X, op=OP.max)
                    ab = wpool.tile([BC, T], f32, tag="ab")
                    nc.vector.tensor_copy(ab[:, 0:16], m4[0:BC, :])
                    nc.gpsimd.tensor_copy(ab[:, 16:32], m4[BC:2 * BC, :])
                    nc.vector.tensor_copy(ab[:, 32:48], m4[2 * BC:3 * BC, :])
                    nc.gpsimd.tensor_copy(ab[:, 48:64], m4[3 * BC:4 * BC, :])
                    anew = wpool.tile([BC, T], f32, tag="alpha")
                    nc.vector.tensor_add(anew[:], ab[:], pc[:, s, :])
                    nc.gpsimd.tensor_copy(hist[BC * tg:BC * (tg + 1), tl, :], anew[:])
                    alpha_tiles[t] = anew

            for tg in range(4):
                nc.gpsimd.dma_start(
                    out=hist_out[:, 256 * tg:256 * (tg + 1), :],
                    in_=hist[BC * tg:BC * (tg + 1), :, :],
                )

    nc.compile()
    _cache["nc"] = nc
    return nc


def _make_tspread(trans):
    # t4[q, jb, i] = trans[i, 16q + jb]
    tt = np.ascontiguousarray(trans.T)  # [j, i]
    return tt.reshape(4, 16, T).astype(np.float32)


def _make_sel():
    sq = np.zeros((4, 128), dtype=np.float32)
    sb = np.zeros((BC, 128), dtype=np.float32)
    for p in range(128):
        sq[p // BC, p] = 1.0
        sb[p % BC, p] = 1.0
    return sq, sb


def kernel(potentials, lengths, transition_params):
    from concourse.bass_utils import run_bass_kernel_spmd

    potentials = np.ascontiguousarray(np.asarray(potentials, dtype=np.float32))
    lengths = np.asarray(lengths, dtype=np.int32)
    trans = np.ascontiguousarray(np.asarray(transition_params, dtype=np.float32))

    nc = _build_program()
    tsp = _make_tspread(trans)
    sq, sb = _make_sel()
    in_maps = [
        {"pots": potentials[c * BC:(c + 1) * BC], "tspread": tsp,
         "selq": sq, "selb": sb}
        for c in range(NCORES)
    ]
    res = run_bass_kernel_spmd(nc, in_maps, core_ids=list(range(NCORES)))
    ah = np.concatenate([res.results[c]["ahist"] for c in range(NCORES)], axis=0)

    # Host backtrack over the device-computed alpha history.
    tags = np.zeros((B, L), dtype=np.int64)
    last = ah[np.arange(B), lengths - 1, :].argmax(axis=1)
    tags[:, L - 1] = last
    lm1 = lengths - 1
    for t in range(L - 2, -1, -1):
        nxt = tags[:, t + 1]
        cand = ah[:, t, :] + trans[:, nxt].T
        tags[:, t] = np.where(t >= lm1, last, cand.argmax(axis=1))
    return tags.astype(np.int32)


# revision 6
# speedup vs baseline: 1.8637x; 1.8637x over previous
"""Viterbi CRF decode on 8 Trainium2 NeuronCores.

Strategy: data-parallel over batch (32 sequences/core). The device kernel runs
the forward max-plus DP (alpha recurrence, the dominant compute) and streams the
full alpha history back to HBM. The host then does the O(L*B*T) backtrack over
that history (0.03% of the FLOPs) plus the sequence-length freeze handling.

Exactness: the device computes alpha_t[j] = max_i(fp32(alpha_{t-1}[i] +
trans[i,j])) + pot_t[j] with the same fp32 rounding as the jax reference, so the
backtrack argmax decisions (first-index tie-break) match bitwise.

Device layout per step (128 partitions = 4 j-quadrants x 32 sequences):
  vt[(q,b), (jb,i)] = alpha[b,i] + trans[i, 16q+jb]   (broadcast add, 1024/partition)
  m4[(q,b), jb]     = max_i vt                        (free-dim reduce)
  alpha'[b, 16q+jb] = m4[(q,b), jb] + pot             (4 collapse copies + add)
"""

import numpy as np

B, L, T = 256, 1024, 64
NCORES = 8
BC = B // NCORES  # 32 sequences per core
CH = 128          # potentials chunk (steps per DMA)

_cache = {}


def _build_program():
    if "nc" in _cache:
        return _cache["nc"]
    import concourse.bacc as bacc
    import concourse.mybir as mybir
    from concourse.tile import TileContext

    f32 = mybir.dt.float32
    AX = mybir.AxisListType
    OP = mybir.AluOpType

    nc = bacc.Bacc("TRN2", target_bir_lowering=False, debug=False)
    pots_in = nc.dram_tensor("pots", [BC, L, T], f32, kind="ExternalInput").ap()
    tsp_in = nc.dram_tensor("tspread", [128, 16, T], f32, kind="ExternalInput").ap()
    hist_out = nc.dram_tensor("ahist", [BC, L, T], f32, kind="ExternalOutput").ap()

    JBD = 7  # jb 0:7 added on DVE, 7:16 on Pool

    with TileContext(nc) as tc:
        with tc.tile_pool(name="const", bufs=1) as cpool, \
             tc.tile_pool(name="pstream", bufs=2) as ppool, \
             tc.tile_pool(name="work", bufs=3) as wpool, \
             tc.tile_pool(name="big", bufs=1) as bpool:
            tsp = cpool.tile([128, 16, T], f32)
            nc.gpsimd.dma_start(out=tsp[:], in_=tsp_in[:])
            hist = bpool.tile([128, 256, T], f32)   # alpha history, 64KB/partition
            arep = cpool.tile([128, T], f32)

            nchunks = L // CH
            for c in range(nchunks):
                pc = ppool.tile([BC, CH, T], f32, tag="pots")
                nc.gpsimd.dma_start(out=pc[:], in_=pots_in[:, c * CH:(c + 1) * CH, :])

                if c == 0:
                    nc.vector.tensor_copy(arep[0:BC, :], pc[:, 0, :])
                    nc.gpsimd.tensor_copy(hist[0:BC, 0, :], arep[0:BC, :])
                    nc.vector.tensor_copy(arep[BC:2 * BC, :], arep[0:BC, :])
                    nc.vector.tensor_copy(arep[2 * BC:4 * BC, :], arep[0:2 * BC, :])

                t0 = max(c * CH, 1)
                for t in range(t0, (c + 1) * CH):
                    tg, tl = t >> 8, t & 255
                    s = t - c * CH
                    # vt[p, jb, i] = alpha[p%32, i] + trans[i, 16*(p//32)+jb]
                    vt = wpool.tile([128, 16, T], f32, tag="vt")
                    nc.vector.tensor_add(
                        vt[:, 0:JBD, :],
                        arep[:].unsqueeze(1).broadcast_to([128, JBD, T]),
                        tsp[:, 0:JBD, :],
                    )
                    nc.gpsimd.tensor_add(
                        vt[:, JBD:16, :],
                        arep[:].unsqueeze(1).broadcast_to([128, 16 - JBD, T]),
                        tsp[:, JBD:16, :],
                    )
                    m4 = wpool.tile([128, 16], f32, tag="m4")
                    nc.vector.tensor_reduce(m4[:], vt[:], axis=AX.X, op=OP.max)
                    ab = wpool.tile([BC, T], f32, tag="ab")
                    nc.vector.tensor_copy(ab[:, 0:16], m4[0:BC, :])
                    nc.gpsimd.tensor_copy(ab[:, 16:32], m4[BC:2 * BC, :])
                    nc.vector.tensor_copy(ab[:, 32:48], m4[2 * BC:3 * BC, :])
                    nc.gpsimd.tensor_copy(ab[:, 48:64], m4[3 * BC:4 * BC, :])
                    nc.vector.tensor_add(arep[0:BC, :], ab[:], pc[:, s, :])
                    nc.gpsimd.tensor_copy(hist[BC * tg:BC * (tg + 1), tl, :], arep[0:BC, :])
                    nc.vector.tensor_copy(arep[BC:2 * BC, :], arep[0:BC, :])
                    nc.vector.tensor_copy(arep[2 * BC:4 * BC, :], arep[0:2 * BC, :])

            for tg in range(4):
                nc.gpsimd.dma_start(
                    out=hist_out[:, 256 * tg:256 * (tg + 1), :],
                    in_=hist[BC * tg:BC * (tg + 1), :, :],
                )

    nc.compile()
    _cache["nc"] = nc
    return nc


def _make_tspread(trans):
    # tsp[32q + b, jb, i] = trans[i, 16q + jb]
    tt = np.ascontiguousarray(trans.T).reshape(4, 16, T)  # [q, jb, i]
    return np.repeat(tt[:, None, :, :], BC, axis=1).reshape(128, 16, T).astype(np.float32)


def kernel(potentials, lengths, transition_params):
    from concourse.bass_utils import run_bass_kernel_spmd

    potentials = np.ascontiguousarray(np.asarray(potentials, dtype=np.float32))
    lengths = np.asarray(lengths, dtype=np.int32)
    trans = np.ascontiguousarray(np.asarray(transition_params, dtype=np.float32))

    nc = _build_program()
    tsp = _make_tspread(trans)
    in_maps = [
        {"pots": potentials[c * BC:(c + 1) * BC], "tspread": tsp}
        for c in range(NCORES)
    ]
    res = run_bass_kernel_spmd(nc, in_maps, core_ids=list(range(NCORES)))
    ah = np.concatenate([res.results[c]["ahist"] for c in range(NCORES)], axis=0)

    # Host backtrack over the device-computed alpha history.
    tags = np.zeros((B, L), dtype=np.int64)
    last = ah[np.arange(B), lengths - 1, :].argmax(axis=1)
    tags[:, L - 1] = last
    lm1 = lengths - 1
    for t in range(L - 2, -1, -1):
        nxt = tags[:, t + 1]
        cand = ah[:, t, :] + trans[:, nxt].T
        tags[:, t] = np.where(t >= lm1, last, cand.argmax(axis=1))
    return tags.astype(np.int32)


# revision 7
# speedup vs baseline: 1.9746x; 1.0595x over previous
"""Viterbi CRF decode on 8 Trainium2 NeuronCores.

Strategy: data-parallel over batch (32 sequences/core). The device kernel runs
the forward max-plus DP (alpha recurrence, the dominant compute) and streams the
full alpha history back to HBM. The host then does the O(L*B*T) backtrack over
that history (0.03% of the FLOPs) plus the sequence-length freeze handling.

Exactness: the device computes alpha_t[j] = max_i(fp32(alpha_{t-1}[i] +
trans[i,j])) + pot_t[j] with the same fp32 rounding as the jax reference, so the
backtrack argmax decisions (first-index tie-break) match bitwise.

Device layout per step (128 partitions = 4 j-quadrants x 32 sequences):
  vt[(q,b), (jb,i)] = alpha[b,i] + trans[i, 16q+jb]   (broadcast add, 1024/partition)
  m4[(q,b), jb]     = max_i vt                        (free-dim reduce)
  alpha'[b, 16q+jb] = m4[(q,b), jb] + pot             (4 collapse copies + add)
"""

import numpy as np

B, L, T = 256, 1024, 64
NCORES = 8
BC = B // NCORES  # 32 sequences per core
CH = 128          # potentials chunk (steps per DMA)

_cache = {}


def _build_program():
    if "nc" in _cache:
        return _cache["nc"]
    import concourse.bacc as bacc
    import concourse.mybir as mybir
    from concourse.tile import TileContext

    f32 = mybir.dt.float32
    AX = mybir.AxisListType
    OP = mybir.AluOpType

    nc = bacc.Bacc("TRN2", target_bir_lowering=False, debug=False)
    pots_in = nc.dram_tensor("pots", [BC, L, T], f32, kind="ExternalInput").ap()
    tsp_in = nc.dram_tensor("tspread", [128, 16, T], f32, kind="ExternalInput").ap()
    hist_out = nc.dram_tensor("ahist", [BC, L, T], f32, kind="ExternalOutput").ap()

    JBD = 12  # jb 0:12 added on DVE, 12:16 on Pool (DVE ~1.07, Pool ~3.0 ns/elem)

    with TileContext(nc) as tc:
        with tc.tile_pool(name="const", bufs=1) as cpool, \
             tc.tile_pool(name="pstream", bufs=2) as ppool, \
             tc.tile_pool(name="work", bufs=3) as wpool, \
             tc.tile_pool(name="big", bufs=1) as bpool:
            tsp = cpool.tile([128, 16, T], f32)
            nc.gpsimd.dma_start(out=tsp[:], in_=tsp_in[:])
            hist = bpool.tile([128, 256, T], f32)   # alpha history, 64KB/partition
            arep = cpool.tile([128, T], f32)

            nchunks = L // CH
            for c in range(nchunks):
                pc = ppool.tile([BC, CH, T], f32, tag="pots")
                nc.gpsimd.dma_start(out=pc[:], in_=pots_in[:, c * CH:(c + 1) * CH, :])

                if c == 0:
                    nc.vector.tensor_copy(arep[0:BC, :], pc[:, 0, :])
                    nc.gpsimd.tensor_copy(hist[0:BC, 0, :], arep[0:BC, :])
                    nc.vector.tensor_copy(arep[BC:2 * BC, :], arep[0:BC, :])
                    nc.vector.tensor_copy(arep[2 * BC:4 * BC, :], arep[0:2 * BC, :])

                t0 = max(c * CH, 1)
                for t in range(t0, (c + 1) * CH):
                    tg, tl = t >> 8, t & 255
                    s = t - c * CH
                    # vt[p, jb, i] = alpha[p%32, i] + trans[i, 16*(p//32)+jb]
                    vt = wpool.tile([128, 16, T], f32, tag="vt")
                    nc.vector.tensor_add(
                        vt[:, 0:JBD, :],
                        arep[:].unsqueeze(1).broadcast_to([128, JBD, T]),
                        tsp[:, 0:JBD, :],
                    )
                    nc.gpsimd.tensor_add(
                        vt[:, JBD:16, :],
                        arep[:].unsqueeze(1).broadcast_to([128, 16 - JBD, T]),
                        tsp[:, JBD:16, :],
                    )
                    m4 = wpool.tile([128, 16], f32, tag="m4")
                    nc.vector.tensor_reduce(m4[:], vt[:], axis=AX.X, op=OP.max)
                    ab = wpool.tile([BC, T], f32, tag="ab")
                    nc.vector.tensor_copy(ab[:, 0:16], m4[0:BC, :])
                    nc.gpsimd.tensor_copy(ab[:, 16:32], m4[BC:2 * BC, :])
                    nc.vector.tensor_copy(ab[:, 32:48], m4[2 * BC:3 * BC, :])
                    nc.gpsimd.tensor_copy(ab[:, 48:64], m4[3 * BC:4 * BC, :])
                    nc.vector.tensor_add(arep[0:BC, :], ab[:], pc[:, s, :])
                    nc.scalar.copy(hist[BC * tg:BC * (tg + 1), tl, :], arep[0:BC, :])
                    nc.vector.tensor_copy(arep[BC:2 * BC, :], arep[0:BC, :])
                    nc.gpsimd.tensor_copy(arep[2 * BC:3 * BC, :], arep[0:BC, :])
                    nc.vector.tensor_copy(arep[3 * BC:4 * BC, :], arep[0:BC, :])

            for tg in range(4):
                nc.gpsimd.dma_start(
                    out=hist_out[:, 256 * tg:256 * (tg + 1), :],
                    in_=hist[BC * tg:BC * (tg + 1), :, :],
                )

    nc.compile()
    _cache["nc"] = nc
    return nc


def _make_tspread(trans):
    # tsp[32q + b, jb, i] = trans[i, 16q + jb]
    tt = np.ascontiguousarray(trans.T).reshape(4, 16, T)  # [q, jb, i]
    return np.repeat(tt[:, None, :, :], BC, axis=1).reshape(128, 16, T).astype(np.float32)


def kernel(potentials, lengths, transition_params):
    from concourse.bass_utils import run_bass_kernel_spmd

    potentials = np.ascontiguousarray(np.asarray(potentials, dtype=np.float32))
    lengths = np.asarray(lengths, dtype=np.int32)
    trans = np.ascontiguousarray(np.asarray(transition_params, dtype=np.float32))

    nc = _build_program()
    tsp = _make_tspread(trans)
    in_maps = [
        {"pots": potentials[c * BC:(c + 1) * BC], "tspread": tsp}
        for c in range(NCORES)
    ]
    res = run_bass_kernel_spmd(nc, in_maps, core_ids=list(range(NCORES)))
    ah = np.concatenate([res.results[c]["ahist"] for c in range(NCORES)], axis=0)

    # Host backtrack over the device-computed alpha history.
    tags = np.zeros((B, L), dtype=np.int64)
    last = ah[np.arange(B), lengths - 1, :].argmax(axis=1)
    tags[:, L - 1] = last
    lm1 = lengths - 1
    for t in range(L - 2, -1, -1):
        nxt = tags[:, t + 1]
        cand = ah[:, t, :] + trans[:, nxt].T
        tags[:, t] = np.where(t >= lm1, last, cand.argmax(axis=1))
    return tags.astype(np.int32)
